# revision 17
# baseline (speedup 1.0000x reference)
"""SAGAN-style self-attention on 8 TRN2 NeuronCores.

Reference computes, per batch b (B=4, C=256, N=64*64=4096, Cq=64):
    q = w_q @ x + b_q            [Cq, N]
    k = w_k @ x + b_k            [Cq, N]
    energy = q^T k               [N, N]
    attention = softmax_j(energy)
    out = gamma * (x @ attention^T) + x
and returns (out, attention).

Sharding: 8 cores = 4 batches x 2 query-row halves (2048 rows each).
Each core computes its [2048, 4096] attention slice and [256, 2048]
output slice; no collectives needed. Host gathers/concatenates.

Per-core device pipeline (16 q-tiles of 128 rows):
  energy tile   : PE matmul fp32r (1 cyc/row), q-tile stationary
  exp + rowsum  : ScalarE activation Exp with accum_out (no max pass;
                  |energy| <~ 6 so exp is fp32-safe unnormalized)
  normalize     : VectorE tensor_scalar (bf16 src, 2x mode) -> f32 attn
  transpose     : PE bf16 transpose of exp values (for the AV contract)
  AV            : PE bf16 matmul, x^T moving; gamma/l folded into epilogue
"""

import sys

sys.path.insert(0, "/opt/trn_rl_repo")

import numpy as np
import ml_dtypes

import concourse.bass as bass
import concourse.bacc as bacc
import concourse.mybir as mybir
from concourse import tile
from concourse.bass_utils import run_bass_kernel_spmd
from concourse.masks import make_identity

B, C, W, H = 4, 256, 64, 64
N = W * H          # 4096 tokens
CQ = C // 4        # 64
NQ = N // 2        # 2048 query rows per core
NT = NQ // 128     # 16 q-tiles per core
NJ = N // 512      # 8 key chunks of 512
NCHUNK = N // 128  # 32 transpose chunks

F32 = mybir.dt.float32
F32R = mybir.dt.float32r
BF16 = mybir.dt.bfloat16
BF16_NP = ml_dtypes.bfloat16

_CACHED = {}


def build_graph(stages="ABCD"):
    import os
    stages = os.environ.get("K_STAGES", stages)
    nc = bacc.Bacc("TRN2", target_bir_lowering=False, debug=False, num_devices=8)

    x_d = nc.dram_tensor("x", [C, N], BF16, kind="ExternalInput").ap()
    xq_d = nc.dram_tensor("xq", [C, NQ], BF16, kind="ExternalInput").ap()
    xt_d = nc.dram_tensor("xt", [128, NCHUNK * C], BF16, kind="ExternalInput").ap()
    xqt_d = nc.dram_tensor("xqt", [128, NT * C], BF16, kind="ExternalInput").ap()
    wqt_d = nc.dram_tensor("wqt", [128, 2 * CQ], BF16, kind="ExternalInput").ap()
    wkt_d = nc.dram_tensor("wkt", [128, 2 * CQ], BF16, kind="ExternalInput").ap()
    bg_d = nc.dram_tensor("bg", [128, 3], F32, kind="ExternalInput").ap()

    attn_d = nc.dram_tensor("attn", [NQ, N], F32, kind="ExternalOutput").ap()
    outt_d = nc.dram_tensor("outt", [NQ, C], F32, kind="ExternalOutput").ap()

    with tile.TileContext(nc) as tc:
        with (
            tc.tile_pool(name="const", bufs=1) as cpool,
            tc.tile_pool(name="kq", bufs=1) as kqpool,
            tc.tile_pool(name="ea", bufs=3) as eapool,
            tc.tile_pool(name="attn", bufs=2) as attnpool,
            tc.tile_pool(name="at", bufs=3) as atpool,
            tc.tile_pool(name="small", bufs=4) as spool,
            tc.tile_pool(name="outsb", bufs=2) as outpool,
        ):
            # ---- constants / full-lifetime inputs ----
            ident = cpool.tile([128, 128], BF16)
            make_identity(nc, ident[:])

            # Scalar-engine DMAs go on the qActDynamicHW ring, parallel to
            # sync's qSPDynamicHW ring — halves prologue DMA issue latency.
            wq_sb = cpool.tile([128, 2, CQ], BF16)
            nc.scalar.dma_start(wq_sb[:], wqt_d.rearrange("p (c o) -> p c o", c=2))
            wk_sb = cpool.tile([128, 2, CQ], BF16)
            nc.scalar.dma_start(wk_sb[:], wkt_d.rearrange("p (c o) -> p c o", c=2))
            bg_sb = cpool.tile([128, 3], F32)
            nc.scalar.dma_start(bg_sb[:], bg_d[:])
            zero_sb = cpool.tile([128, 1], F32)
            nc.gpsimd.memset(zero_sb[:], 0.0)
            # Preload the Exp table off the critical path.
            scratch1 = cpool.tile([128, 1], BF16)
            nc.scalar.activation(
                scratch1[:], zero_sb[:, 0:1],
                mybir.ActivationFunctionType.Exp, bias=zero_sb[:, 0:1],
            )

            x_sb = [cpool.tile([128, N], BF16, tag=f"x{i}", name=f"x_sb{i}") for i in range(2)]
            xq_sb = [cpool.tile([128, NQ], BF16, tag=f"xq{i}", name=f"xq_sb{i}") for i in range(2)]

            last_in_dma = [None]

            def load_x(ci, q):
                last_in_dma[0] = nc.sync.dma_start(
                    x_sb[ci][:, q * 1024 : (q + 1) * 1024],
                    x_d[ci * 128 : (ci + 1) * 128, q * 1024 : (q + 1) * 1024],
                )

            def load_xq(ci, hh):
                nc.scalar.dma_start(
                    xq_sb[ci][:, hh * 1024 : (hh + 1) * 1024],
                    xq_d[ci * 128 : (ci + 1) * 128, hh * 1024 : (hh + 1) * 1024],
                )

            # Need-order: q-proj chunk 0 (xq h0) and k-proj 0/1 (x q0) gate
            # tile-0 energy; everything else trails.
            load_xq(0, 0); load_xq(1, 0)
            load_x(0, 0); load_x(1, 0)
            load_xq(0, 1); load_xq(1, 1)
            load_x(0, 1); load_x(1, 1)
            load_x(0, 2); load_x(1, 2)
            load_x(0, 3); load_x(1, 3)

            # ---- projections + tile-0 energy, interleaved in need-order ----
            k_sb = kqpool.tile([CQ, N], BF16)
            q_sb = kqpool.tile([CQ, NQ], BF16)
            with (
                tc.tile_pool(name="e_ps", bufs=2, space="PSUM") as e_ps,
                tc.tile_pool(name="t_ps", bufs=2, space="PSUM") as t_ps,
                tc.tile_pool(name="o_ps", bufs=2, space="PSUM") as o_ps,
            ):
                proj_ps = o_ps  # share the AV pool's banks (free in prologue)
                def proj_chunk(w_sb, src_sb, dst, jc, bias_col, name):
                    ps = proj_ps.tile([CQ, 512], F32, tag="av", name=name)
                    for cc in range(2):
                        nc.tensor.matmul(
                            ps[:],
                            w_sb[:, cc, :],
                            src_sb[cc][:, jc * 512 : (jc + 1) * 512],
                            start=(cc == 0),
                            stop=(cc == 1),
                        )
                    nc.vector.tensor_scalar(
                        out=dst[:, jc * 512 : (jc + 1) * 512],
                        in0=ps[:],
                        scalar1=bg_sb[0:CQ, bias_col : bias_col + 1],
                        scalar2=None,
                        op0=mybir.AluOpType.add,
                    )

                def energy_quarter(t, qi, ea, lpart):
                    qs = t * 128
                    eps = e_ps.tile([128, 1024], F32, tag="energy", name=f"e{t}_{qi}")
                    for jj in range(2):
                        jc = qi * 2 + jj
                        nc.tensor.matmul(
                            eps[:, jj * 512 : (jj + 1) * 512],
                            q_sb[:, qs : qs + 128],
                            k_sb[:, jc * 512 : (jc + 1) * 512],
                            start=True,
                            stop=True,
                        )
                    nc.scalar.activation(
                        ea[:, qi * 1024 : (qi + 1) * 1024],
                        eps[:],
                        mybir.ActivationFunctionType.Exp,
                        bias=zero_sb[:, 0:1],
                        accum_out=lpart[:, qi : qi + 1],
                    )

                # tile-0 ea/lpart allocated up front so its energy quarters can
                # be emitted inside the projection sequence
                ea0 = eapool.tile([128, N], BF16, tag="ea", name="ea_t0")
                lpart0 = spool.tile([128, 4], F32, tag="lpart", name="lpart_t0")
                proj_chunk(wq_sb, xq_sb, q_sb, 0, 0, "qproj0")
                for g in range(4):
                    proj_chunk(wk_sb, x_sb, k_sb, 2 * g, 1, f"kproj{2*g}")
                    proj_chunk(wk_sb, x_sb, k_sb, 2 * g + 1, 1, f"kproj{2*g+1}")
                    if g > 0:
                        proj_chunk(wq_sb, xq_sb, q_sb, g, 0, f"qproj{g}")
                    energy_quarter(0, g, ea0, lpart0)

                # xt/xqt are first needed by tile-0 AV; chain them behind the
                # x loads so they do not steal early DMA bandwidth.
                xt_sb = cpool.tile([128, NCHUNK, C], BF16)
                i1 = nc.sync.dma_start(
                    xt_sb[:], xt_d.rearrange("p (j c) -> p j c", j=NCHUNK)
                )
                bass._add_dep_helper(i1.ins, last_in_dma[0].ins, False,
                                     "xt load after x loads")
                xqt_sb = cpool.tile([128, NT, C], BF16)
                i2 = nc.scalar.dma_start(
                    xqt_sb[:], xqt_d.rearrange("p (t c) -> p t c", t=NT)
                )

                # ---- main loop over q-tiles ----
                for t in range(NT):
                    qs = t * 128
                    if t == 0:
                        ea, lpart = ea0, lpart0
                    else:
                        ea = eapool.tile([128, N], BF16, tag="ea", name=f"ea_t{t}")
                        lpart = spool.tile([128, 4], F32, tag="lpart",
                                           name=f"lpart_t{t}")
                        for qi in range(4):
                            energy_quarter(t, qi, ea, lpart)
                    # transpose exp values (bf16) for the AV contraction.
                    # Emitted before the normalize so the DVE runs the copies
                    # (which gate PE's AV matmuls) ahead of the long normalize.
                    at_sb = atpool.tile([128, NCHUNK * 128], BF16, tag="at")
                    for g in range(4):
                        tps = t_ps.tile([128, 1024], BF16, tag="tr")
                        for i in range(8):
                            j = g * 8 + i
                            nc.tensor.transpose(
                                tps[:, i * 128 : (i + 1) * 128],
                                ea[:, j * 128 : (j + 1) * 128],
                                ident[:],
                            )
                        nc.vector.tensor_copy(
                            at_sb[:, g * 1024 : (g + 1) * 1024], tps[:]
                        )
                    # AV: out~^T[m, c] = sum_n A~T[n, m] x^T[n, c]
                    ops = o_ps.tile([128, C], F32, tag="av")
                    for j in range(NCHUNK):
                        nc.tensor.matmul(
                            ops[:],
                            at_sb[:, j * 128 : (j + 1) * 128],
                            xt_sb[:, j, :],
                            start=(j == 0),
                            stop=(j == NCHUNK - 1),
                        )
                    # softmax scales
                    l_sum = spool.tile([128, 1], F32, tag="lsum")
                    nc.vector.reduce_sum(l_sum[:], lpart[:], axis=mybir.AxisListType.X)
                    inv_l = spool.tile([128, 1], F32, tag="invl")
                    nc.vector.reciprocal(inv_l[:], l_sum[:])
                    ginv = spool.tile([128, 1], F32, tag="ginv")
                    nc.vector.tensor_mul(ginv[:], inv_l[:], bg_sb[:, 2:3])
                    # normalized attention row block -> DRAM
                    attn_sb = attnpool.tile([128, N], F32, tag="attn")
                    nc.vector.tensor_scalar(
                        out=attn_sb[:],
                        in0=ea[:],
                        scalar1=inv_l[:, 0:1],
                        scalar2=None,
                        op0=mybir.AluOpType.mult,
                    )
                    nc.sync.dma_start(attn_d[qs : qs + 128, :], attn_sb[:])
                    # epilogue: outt = (gamma/l) * out~^T + xq^T
                    outt_sb = outpool.tile([128, C], F32, tag="outt")
                    av_bf = outpool.tile([128, C], BF16, tag="avbf")
                    # on ScalarE: frees the AV PSUM slot without queuing behind
                    # the normalize on the vector engine
                    nc.scalar.activation(
                        av_bf[:],
                        ops[:],
                        mybir.ActivationFunctionType.Copy,
                        bias=0.0,
                        scale=ginv[:, 0:1],
                    )
                    nc.vector.tensor_add(outt_sb[:], av_bf[:], xqt_sb[:, t, :])
                    nc.sync.dma_start(outt_d[qs : qs + 128, :], outt_sb[:])

    nc.compile()
    return nc


def _prep_inputs(x, w_q, b_q, w_k, b_k, gamma):
    xf = np.ascontiguousarray(x.reshape(B, C, N)).astype(np.float32)
    wqt = np.ascontiguousarray(w_q.T)  # [C, CQ]
    wkt = np.ascontiguousarray(w_k.T)
    # [128, 2*CQ]: row p holds wqt[p, :] then wqt[p+128, :]
    wqt_r = wqt.reshape(2, 128, CQ).transpose(1, 0, 2).reshape(128, 2 * CQ)
    wkt_r = wkt.reshape(2, 128, CQ).transpose(1, 0, 2).reshape(128, 2 * CQ)
    wqt_r = np.ascontiguousarray(wqt_r).astype(BF16_NP)
    wkt_r = np.ascontiguousarray(wkt_r).astype(BF16_NP)
    bg = np.zeros((128, 3), np.float32)
    bg[:CQ, 0] = b_q
    bg[:CQ, 1] = b_k
    bg[:, 2] = gamma[0]

    in_maps = []
    for b in range(B):
        xb = xf[b]  # [C, N]
        xtb = np.ascontiguousarray(xb.T)  # [N, C] f32
        xt_r = (
            xtb.reshape(NCHUNK, 128, C)
            .transpose(1, 0, 2)
            .reshape(128, NCHUNK * C)
            .astype(BF16_NP)
        )
        for h in range(2):
            xq = np.ascontiguousarray(xb[:, h * NQ : (h + 1) * NQ])
            xqt = np.ascontiguousarray(xq.T)  # [NQ, C]
            xqt_r = (
                xqt.reshape(NT, 128, C)
                .transpose(1, 0, 2)
                .reshape(128, NT * C)
                .astype(BF16_NP)
            )
            in_maps.append(
                {
                    "x": xb.astype(BF16_NP),
                    "xq": xq.astype(BF16_NP),
                    "xt": xt_r,
                    "xqt": xqt_r,
                    "wqt": wqt_r,
                    "wkt": wkt_r,
                    "bg": bg,
                }
            )
    return in_maps


def run_on_device(x, w_q, b_q, w_k, b_k, gamma, trace=False, tmpdir=None):
    if "nc" not in _CACHED:
        _CACHED["nc"] = build_graph()
    nc = _CACHED["nc"]
    in_maps = _prep_inputs(x, w_q, b_q, w_k, b_k, gamma)
    res = run_bass_kernel_spmd(
        nc, in_maps, core_ids=list(range(8)), trace=trace, tmpdir=tmpdir
    )
    out = np.empty((B, C, N), np.float32)
    attention = np.empty((B, N, N), np.float32)
    for core in range(8):
        b, h = divmod(core, 2)
        r = res.results[core]
        attention[b, h * NQ : (h + 1) * NQ, :] = r["attn"]
        out[b][:, h * NQ : (h + 1) * NQ] = r["outt"].T
    return out.reshape(B, C, W, H), attention, res


def kernel(x, w_q, b_q, w_k, b_k, gamma):
    out, attention, _ = run_on_device(x, w_q, b_q, w_k, b_k, gamma)
    return out, attention


# revision 18
# speedup vs baseline: 1.0753x; 1.0753x over previous
"""SAGAN-style self-attention on 8 TRN2 NeuronCores.

Reference computes, per batch b (B=4, C=256, N=64*64=4096, Cq=64):
    q = w_q @ x + b_q            [Cq, N]
    k = w_k @ x + b_k            [Cq, N]
    energy = q^T k               [N, N]
    attention = softmax_j(energy)
    out = gamma * (x @ attention^T) + x
and returns (out, attention).

Sharding: 8 cores = 4 batches x 2 query-row halves (2048 rows each).
Each core computes its [2048, 4096] attention slice and [256, 2048]
output slice; no collectives needed. Host gathers/concatenates.

Per-core device pipeline (16 q-tiles of 128 rows):
  energy tile   : PE matmul fp32r (1 cyc/row), q-tile stationary
  exp + rowsum  : ScalarE activation Exp with accum_out (no max pass;
                  |energy| <~ 6 so exp is fp32-safe unnormalized)
  normalize     : VectorE tensor_scalar (bf16 src, 2x mode) -> f32 attn
  transpose     : PE bf16 transpose of exp values (for the AV contract)
  AV            : PE bf16 matmul, x^T moving; gamma/l folded into epilogue
"""

import sys

sys.path.insert(0, "/opt/trn_rl_repo")

import numpy as np
import ml_dtypes

import concourse.bass as bass
import concourse.bacc as bacc
import concourse.mybir as mybir
from concourse import tile
from concourse.bass_utils import run_bass_kernel_spmd
from concourse.masks import make_identity

B, C, W, H = 4, 256, 64, 64
N = W * H          # 4096 tokens
CQ = C // 4        # 64
NQ = N // 2        # 2048 query rows per core
NT = NQ // 128     # 16 q-tiles per core
NJ = N // 512      # 8 key chunks of 512
NCHUNK = N // 128  # 32 transpose chunks

F32 = mybir.dt.float32
F32R = mybir.dt.float32r
BF16 = mybir.dt.bfloat16
BF16_NP = ml_dtypes.bfloat16

_CACHED = {}


def build_graph(stages="ABCD"):
    import os
    stages = os.environ.get("K_STAGES", stages)
    nc = bacc.Bacc("TRN2", target_bir_lowering=False, debug=False, num_devices=8)

    x_d = nc.dram_tensor("x", [C, N], BF16, kind="ExternalInput").ap()
    xq_d = nc.dram_tensor("xq", [C, NQ], BF16, kind="ExternalInput").ap()
    xt_d = nc.dram_tensor("xt", [128, NCHUNK * C], BF16, kind="ExternalInput").ap()
    xqt_d = nc.dram_tensor("xqt", [128, NT * C], BF16, kind="ExternalInput").ap()
    wqt_d = nc.dram_tensor("wqt", [128, 2 * CQ], BF16, kind="ExternalInput").ap()
    wkt_d = nc.dram_tensor("wkt", [128, 2 * CQ], BF16, kind="ExternalInput").ap()
    bg_d = nc.dram_tensor("bg", [128, 3], F32, kind="ExternalInput").ap()

    attn_d = nc.dram_tensor("attn", [NQ, N], F32, kind="ExternalOutput").ap()
    outt_d = nc.dram_tensor("outt", [NQ, C], F32, kind="ExternalOutput").ap()

    with tile.TileContext(nc) as tc:
        with (
            tc.tile_pool(name="const", bufs=1) as cpool,
            tc.tile_pool(name="kq", bufs=1) as kqpool,
            tc.tile_pool(name="ea", bufs=3) as eapool,
            tc.tile_pool(name="attn", bufs=2) as attnpool,
            tc.tile_pool(name="at", bufs=3) as atpool,
            tc.tile_pool(name="small", bufs=4) as spool,
            tc.tile_pool(name="outsb", bufs=2) as outpool,
        ):
            # ---- constants / full-lifetime inputs ----
            ident = cpool.tile([128, 128], BF16)
            make_identity(nc, ident[:])

            # Scalar-engine DMAs go on the qActDynamicHW ring, parallel to
            # sync's qSPDynamicHW ring — halves prologue DMA issue latency.
            wq_sb = cpool.tile([128, 2, CQ], BF16)
            nc.scalar.dma_start(wq_sb[:], wqt_d.rearrange("p (c o) -> p c o", c=2))
            wk_sb = cpool.tile([128, 2, CQ], BF16)
            nc.scalar.dma_start(wk_sb[:], wkt_d.rearrange("p (c o) -> p c o", c=2))
            bg_sb = cpool.tile([128, 3], F32)
            nc.scalar.dma_start(bg_sb[:], bg_d[:])
            zero_sb = cpool.tile([128, 1], F32)
            nc.gpsimd.memset(zero_sb[:], 0.0)
            # Preload the Exp table off the critical path.
            scratch1 = cpool.tile([128, 1], BF16)
            nc.scalar.activation(
                scratch1[:], zero_sb[:, 0:1],
                mybir.ActivationFunctionType.Exp, bias=zero_sb[:, 0:1],
            )

            x_sb = [cpool.tile([128, N], BF16, tag=f"x{i}", name=f"x_sb{i}") for i in range(2)]
            xq_sb = [cpool.tile([128, NQ], BF16, tag=f"xq{i}", name=f"xq_sb{i}") for i in range(2)]

            last_in_dma = [None]

            def load_x(ci, q):
                last_in_dma[0] = nc.sync.dma_start(
                    x_sb[ci][:, q * 1024 : (q + 1) * 1024],
                    x_d[ci * 128 : (ci + 1) * 128, q * 1024 : (q + 1) * 1024],
                )

            def load_xq(ci, hh):
                nc.scalar.dma_start(
                    xq_sb[ci][:, hh * 1024 : (hh + 1) * 1024],
                    xq_d[ci * 128 : (ci + 1) * 128, hh * 1024 : (hh + 1) * 1024],
                )

            # Need-order: q-proj chunk 0 (xq h0) and k-proj 0/1 (x q0) gate
            # tile-0 energy; everything else trails.
            load_xq(0, 0); load_xq(1, 0)
            load_x(0, 0); load_x(1, 0)
            load_xq(0, 1); load_xq(1, 1)
            load_x(0, 1); load_x(1, 1)
            load_x(0, 2); load_x(1, 2)
            load_x(0, 3); load_x(1, 3)

            # ---- projections + tile-0 energy, interleaved in need-order ----
            k_sb = kqpool.tile([CQ, N], BF16)
            q_sb = kqpool.tile([CQ, NQ], BF16)
            with (
                tc.tile_pool(name="e_ps", bufs=2, space="PSUM") as e_ps,
                tc.tile_pool(name="t_ps", bufs=2, space="PSUM") as t_ps,
                tc.tile_pool(name="o_ps", bufs=2, space="PSUM") as o_ps,
            ):
                proj_ps = o_ps  # share the AV pool's banks (free in prologue)
                def proj_chunk(w_sb, src_sb, dst, jc, bias_col, name):
                    ps = proj_ps.tile([CQ, 512], F32, tag="av", name=name)
                    for cc in range(2):
                        nc.tensor.matmul(
                            ps[:],
                            w_sb[:, cc, :],
                            src_sb[cc][:, jc * 512 : (jc + 1) * 512],
                            start=(cc == 0),
                            stop=(cc == 1),
                        )
                    nc.vector.tensor_scalar(
                        out=dst[:, jc * 512 : (jc + 1) * 512],
                        in0=ps[:],
                        scalar1=bg_sb[0:CQ, bias_col : bias_col + 1],
                        scalar2=None,
                        op0=mybir.AluOpType.add,
                    )

                def energy_quarter(t, qi, ea, lpart):
                    qs = t * 128
                    eps = e_ps.tile([128, 1024], F32, tag="energy", name=f"e{t}_{qi}")
                    for jj in range(2):
                        jc = qi * 2 + jj
                        nc.tensor.matmul(
                            eps[:, jj * 512 : (jj + 1) * 512],
                            q_sb[:, qs : qs + 128],
                            k_sb[:, jc * 512 : (jc + 1) * 512],
                            start=True,
                            stop=True,
                        )
                    nc.scalar.activation(
                        ea[:, qi * 1024 : (qi + 1) * 1024],
                        eps[:],
                        mybir.ActivationFunctionType.Exp,
                        bias=zero_sb[:, 0:1],
                        accum_out=lpart[:, qi : qi + 1],
                    )

                # tile-0 ea/lpart allocated up front so its energy quarters can
                # be emitted inside the projection sequence
                ea0 = eapool.tile([128, N], BF16, tag="ea", name="ea_t0")
                lpart0 = spool.tile([128, 4], F32, tag="lpart", name="lpart_t0")
                proj_chunk(wq_sb, xq_sb, q_sb, 0, 0, "qproj0")
                for g in range(4):
                    proj_chunk(wk_sb, x_sb, k_sb, 2 * g, 1, f"kproj{2*g}")
                    proj_chunk(wk_sb, x_sb, k_sb, 2 * g + 1, 1, f"kproj{2*g+1}")
                    if g > 0:
                        proj_chunk(wq_sb, xq_sb, q_sb, g, 0, f"qproj{g}")
                    energy_quarter(0, g, ea0, lpart0)

                # xt/xqt are first needed by tile-0 AV; chain them behind the
                # x loads so they do not steal early DMA bandwidth.
                xt_sb = cpool.tile([128, NCHUNK, C], BF16)
                i1 = nc.sync.dma_start(
                    xt_sb[:], xt_d.rearrange("p (j c) -> p j c", j=NCHUNK)
                )
                bass._add_dep_helper(i1.ins, last_in_dma[0].ins, False,
                                     "xt load after x loads")
                xqt_sb = cpool.tile([128, NT, C], BF16)
                i2 = nc.scalar.dma_start(
                    xqt_sb[:], xqt_d.rearrange("p (t c) -> p t c", t=NT)
                )

                # ---- main loop over q-tiles, software-pipelined ----
                # PE stream per iteration: T(t) -> E(t+1) -> AV(t); the E(t+1)
                # matmuls give the DVE copies of T(t) time to land before
                # AV(t) needs them.
                eas = {0: (ea0, lpart0)}

                def emit_energy_tile(t):
                    ea = eapool.tile([128, N], BF16, tag="ea", name=f"ea_t{t}")
                    lpart = spool.tile([128, 4], F32, tag="lpart",
                                       name=f"lpart_t{t}")
                    for qi in range(4):
                        energy_quarter(t, qi, ea, lpart)
                    eas[t] = (ea, lpart)

                emit_energy_tile(1)
                for t in range(NT):
                    qs = t * 128
                    ea, lpart = eas.pop(t)
                    # transpose exp values (bf16) for the AV contraction.
                    # Emitted before the normalize so the DVE runs the copies
                    # (which gate PE's AV matmuls) ahead of the long normalize.
                    at_sb = atpool.tile([128, NCHUNK * 128], BF16, tag="at")
                    for g in range(4):
                        tps = t_ps.tile([128, 1024], BF16, tag="tr")
                        for i in range(8):
                            j = g * 8 + i
                            nc.tensor.transpose(
                                tps[:, i * 128 : (i + 1) * 128],
                                ea[:, j * 128 : (j + 1) * 128],
                                ident[:],
                            )
                        nc.vector.tensor_copy(
                            at_sb[:, g * 1024 : (g + 1) * 1024], tps[:]
                        )
                    if t + 2 < NT:
                        emit_energy_tile(t + 2)
                    # AV: out~^T[m, c] = sum_n A~T[n, m] x^T[n, c]
                    ops = o_ps.tile([128, C], F32, tag="av")
                    for j in range(NCHUNK):
                        nc.tensor.matmul(
                            ops[:],
                            at_sb[:, j * 128 : (j + 1) * 128],
                            xt_sb[:, j, :],
                            start=(j == 0),
                            stop=(j == NCHUNK - 1),
                        )
                    # softmax scales
                    l_sum = spool.tile([128, 1], F32, tag="lsum")
                    nc.vector.reduce_sum(l_sum[:], lpart[:], axis=mybir.AxisListType.X)
                    inv_l = spool.tile([128, 1], F32, tag="invl")
                    nc.vector.reciprocal(inv_l[:], l_sum[:])
                    ginv = spool.tile([128, 1], F32, tag="ginv")
                    nc.vector.tensor_mul(ginv[:], inv_l[:], bg_sb[:, 2:3])
                    # normalized attention row block -> DRAM
                    attn_sb = attnpool.tile([128, N], F32, tag="attn")
                    nc.vector.tensor_scalar(
                        out=attn_sb[:],
                        in0=ea[:],
                        scalar1=inv_l[:, 0:1],
                        scalar2=None,
                        op0=mybir.AluOpType.mult,
                    )
                    nc.sync.dma_start(attn_d[qs : qs + 128, :], attn_sb[:])
                    # epilogue: outt = (gamma/l) * out~^T + xq^T
                    outt_sb = outpool.tile([128, C], F32, tag="outt")
                    av_bf = outpool.tile([128, C], BF16, tag="avbf")
                    # on ScalarE: frees the AV PSUM slot without queuing behind
                    # the normalize on the vector engine
                    nc.scalar.activation(
                        av_bf[:],
                        ops[:],
                        mybir.ActivationFunctionType.Copy,
                        bias=0.0,
                        scale=ginv[:, 0:1],
                    )
                    nc.vector.tensor_add(outt_sb[:], av_bf[:], xqt_sb[:, t, :])
                    nc.sync.dma_start(outt_d[qs : qs + 128, :], outt_sb[:])

    nc.compile()
    return nc


def _prep_inputs(x, w_q, b_q, w_k, b_k, gamma):
    xf = np.ascontiguousarray(x.reshape(B, C, N)).astype(np.float32)
    wqt = np.ascontiguousarray(w_q.T)  # [C, CQ]
    wkt = np.ascontiguousarray(w_k.T)
    # [128, 2*CQ]: row p holds wqt[p, :] then wqt[p+128, :]
    wqt_r = wqt.reshape(2, 128, CQ).transpose(1, 0, 2).reshape(128, 2 * CQ)
    wkt_r = wkt.reshape(2, 128, CQ).transpose(1, 0, 2).reshape(128, 2 * CQ)
    wqt_r = np.ascontiguousarray(wqt_r).astype(BF16_NP)
    wkt_r = np.ascontiguousarray(wkt_r).astype(BF16_NP)
    bg = np.zeros((128, 3), np.float32)
    bg[:CQ, 0] = b_q
    bg[:CQ, 1] = b_k
    bg[:, 2] = gamma[0]

    in_maps = []
    for b in range(B):
        xb = xf[b]  # [C, N]
        xtb = np.ascontiguousarray(xb.T)  # [N, C] f32
        xt_r = (
            xtb.reshape(NCHUNK, 128, C)
            .transpose(1, 0, 2)
            .reshape(128, NCHUNK * C)
            .astype(BF16_NP)
        )
        for h in range(2):
            xq = np.ascontiguousarray(xb[:, h * NQ : (h + 1) * NQ])
            xqt = np.ascontiguousarray(xq.T)  # [NQ, C]
            xqt_r = (
                xqt.reshape(NT, 128, C)
                .transpose(1, 0, 2)
                .reshape(128, NT * C)
                .astype(BF16_NP)
            )
            in_maps.append(
                {
                    "x": xb.astype(BF16_NP),
                    "xq": xq.astype(BF16_NP),
                    "xt": xt_r,
                    "xqt": xqt_r,
                    "wqt": wqt_r,
                    "wkt": wkt_r,
                    "bg": bg,
                }
            )
    return in_maps


def run_on_device(x, w_q, b_q, w_k, b_k, gamma, trace=False, tmpdir=None):
    if "nc" not in _CACHED:
        _CACHED["nc"] = build_graph()
    nc = _CACHED["nc"]
    in_maps = _prep_inputs(x, w_q, b_q, w_k, b_k, gamma)
    res = run_bass_kernel_spmd(
        nc, in_maps, core_ids=list(range(8)), trace=trace, tmpdir=tmpdir
    )
    out = np.empty((B, C, N), np.float32)
    attention = np.empty((B, N, N), np.float32)
    for core in range(8):
        b, h = divmod(core, 2)
        r = res.results[core]
        attention[b, h * NQ : (h + 1) * NQ, :] = r["attn"]
        out[b][:, h * NQ : (h + 1) * NQ] = r["outt"].T
    return out.reshape(B, C, W, H), attention, res


def kernel(x, w_q, b_q, w_k, b_k, gamma):
    out, attention, _ = run_on_device(x, w_q, b_q, w_k, b_k, gamma)
    return out, attention


# revision 19
# speedup vs baseline: 1.1243x; 1.0455x over previous
"""SAGAN-style self-attention on 8 TRN2 NeuronCores.

Reference computes, per batch b (B=4, C=256, N=64*64=4096, Cq=64):
    q = w_q @ x + b_q            [Cq, N]
    k = w_k @ x + b_k            [Cq, N]
    energy = q^T k               [N, N]
    attention = softmax_j(energy)
    out = gamma * (x @ attention^T) + x
and returns (out, attention).

Sharding: 8 cores = 4 batches x 2 query-row halves (2048 rows each).
Each core computes its [2048, 4096] attention slice and [256, 2048]
output slice; no collectives needed. Host gathers/concatenates.

Per-core device pipeline (16 q-tiles of 128 rows):
  energy tile   : PE matmul fp32r (1 cyc/row), q-tile stationary
  exp + rowsum  : ScalarE activation Exp with accum_out (no max pass;
                  |energy| <~ 6 so exp is fp32-safe unnormalized)
  normalize     : VectorE tensor_scalar (bf16 src, 2x mode) -> f32 attn
  transpose     : PE bf16 transpose of exp values (for the AV contract)
  AV            : PE bf16 matmul, x^T moving; gamma/l folded into epilogue
"""

import sys

sys.path.insert(0, "/opt/trn_rl_repo")

import numpy as np
import ml_dtypes

import concourse.bass as bass
import concourse.bacc as bacc
import concourse.mybir as mybir
from concourse import tile
from concourse.bass_utils import run_bass_kernel_spmd
from concourse.masks import make_identity

B, C, W, H = 4, 256, 64, 64
N = W * H          # 4096 tokens
CQ = C // 4        # 64
NQ = N // 2        # 2048 query rows per core
NT = NQ // 128     # 16 q-tiles per core
NJ = N // 512      # 8 key chunks of 512
NCHUNK = N // 128  # 32 transpose chunks

F32 = mybir.dt.float32
F32R = mybir.dt.float32r
BF16 = mybir.dt.bfloat16
BF16_NP = ml_dtypes.bfloat16

_CACHED = {}


def build_graph(stages="ABCD"):
    import os
    stages = os.environ.get("K_STAGES", stages)
    nc = bacc.Bacc("TRN2", target_bir_lowering=False, debug=False, num_devices=8)

    x_d = nc.dram_tensor("x", [C, N], BF16, kind="ExternalInput").ap()
    xq_d = nc.dram_tensor("xq", [C, NQ], BF16, kind="ExternalInput").ap()
    xt_d = nc.dram_tensor("xt", [128, NCHUNK * C], BF16, kind="ExternalInput").ap()
    xqt_d = nc.dram_tensor("xqt", [128, NT * C], BF16, kind="ExternalInput").ap()
    wqt_d = nc.dram_tensor("wqt", [128, 4 * CQ], BF16, kind="ExternalInput").ap()
    wkt_d = nc.dram_tensor("wkt", [128, 4 * CQ], BF16, kind="ExternalInput").ap()
    bg_d = nc.dram_tensor("bg", [128, 3], F32, kind="ExternalInput").ap()

    attn_d = nc.dram_tensor("attn", [NQ, N], F32, kind="ExternalOutput").ap()
    outt_d = nc.dram_tensor("outt", [NQ, C], F32, kind="ExternalOutput").ap()

    with tile.TileContext(nc) as tc:
        with (
            tc.tile_pool(name="const", bufs=1) as cpool,
            tc.tile_pool(name="kq", bufs=1) as kqpool,
            tc.tile_pool(name="ea", bufs=3) as eapool,
            tc.tile_pool(name="attn", bufs=2) as attnpool,
            tc.tile_pool(name="at", bufs=3) as atpool,
            tc.tile_pool(name="small", bufs=4) as spool,
            tc.tile_pool(name="outsb", bufs=2) as outpool,
        ):
            # ---- constants / full-lifetime inputs ----
            ident = cpool.tile([128, 128], BF16)
            make_identity(nc, ident[:])

            # Scalar-engine DMAs go on the qActDynamicHW ring, parallel to
            # sync's qSPDynamicHW ring — halves prologue DMA issue latency.
            wq_sb = cpool.tile([128, 2, 2 * CQ], BF16)
            nc.scalar.dma_start(wq_sb[:], wqt_d.rearrange("p (c o) -> p c o", c=2))
            wk_sb = cpool.tile([128, 2, 2 * CQ], BF16)
            nc.scalar.dma_start(wk_sb[:], wkt_d.rearrange("p (c o) -> p c o", c=2))
            bg_sb = cpool.tile([128, 3], F32)
            nc.scalar.dma_start(bg_sb[:], bg_d[:])
            zero_sb = cpool.tile([128, 1], F32)
            nc.gpsimd.memset(zero_sb[:], 0.0)
            # Preload the Exp table off the critical path.
            scratch1 = cpool.tile([128, 1], BF16)
            nc.scalar.activation(
                scratch1[:], zero_sb[:, 0:1],
                mybir.ActivationFunctionType.Exp, bias=zero_sb[:, 0:1],
            )

            x_sb = [cpool.tile([128, N], BF16, tag=f"x{i}", name=f"x_sb{i}") for i in range(2)]
            xq_sb = [cpool.tile([128, NQ], BF16, tag=f"xq{i}", name=f"xq_sb{i}") for i in range(2)]

            last_in_dma = [None]

            def load_x(ci, q):
                last_in_dma[0] = nc.sync.dma_start(
                    x_sb[ci][:, q * 1024 : (q + 1) * 1024],
                    x_d[ci * 128 : (ci + 1) * 128, q * 1024 : (q + 1) * 1024],
                )

            def load_xq(ci, hh):
                nc.scalar.dma_start(
                    xq_sb[ci][:, hh * 1024 : (hh + 1) * 1024],
                    xq_d[ci * 128 : (ci + 1) * 128, hh * 1024 : (hh + 1) * 1024],
                )

            # Need-order: q-proj chunk 0 (xq h0) and k-proj 0/1 (x q0) gate
            # tile-0 energy; everything else trails.
            load_xq(0, 0); load_xq(1, 0)
            load_x(0, 0); load_x(1, 0)
            load_xq(0, 1); load_xq(1, 1)
            load_x(0, 1); load_x(1, 1)
            load_x(0, 2); load_x(1, 2)
            load_x(0, 3); load_x(1, 3)

            # ---- projections + tile-0 energy, interleaved in need-order ----
            # k/q live duplicated in both partition halves (rows 0-63 and
            # 64-127) so consecutive energy matmuls can alternate PE
            # row-groups, letting LDWEIGHTS overlap in-flight matmuls.
            k_sb = kqpool.tile([128, N], BF16)
            q_sb = kqpool.tile([128, NQ], BF16)
            with (
                tc.tile_pool(name="e_ps", bufs=2, space="PSUM") as e_ps,
                tc.tile_pool(name="t_ps", bufs=2, space="PSUM") as t_ps,
                tc.tile_pool(name="o_ps", bufs=2, space="PSUM") as o_ps,
            ):
                proj_ps = o_ps  # share the AV pool's banks (free in prologue)
                def proj_chunk(w_sb, src_sb, dst, jc, bias_col, name):
                    ps = proj_ps.tile([128, 512], F32, tag="av", name=name)
                    for cc in range(2):
                        nc.tensor.matmul(
                            ps[:],
                            w_sb[:, cc, :],
                            src_sb[cc][:, jc * 512 : (jc + 1) * 512],
                            start=(cc == 0),
                            stop=(cc == 1),
                        )
                    nc.vector.tensor_scalar(
                        out=dst[:, jc * 512 : (jc + 1) * 512],
                        in0=ps[:],
                        scalar1=bg_sb[:, bias_col : bias_col + 1],
                        scalar2=None,
                        op0=mybir.AluOpType.add,
                    )

                def energy_quarter(t, qi, ea, lpart):
                    qs = t * 128
                    eps = e_ps.tile([128, 1024], F32, tag="energy", name=f"e{t}_{qi}")
                    for jj in range(2):
                        jc = qi * 2 + jj
                        rh = slice(0, CQ) if jc % 2 == 0 else slice(CQ, 128)
                        nc.tensor.matmul(
                            eps[:, jj * 512 : (jj + 1) * 512],
                            q_sb[rh, qs : qs + 128],
                            k_sb[rh, jc * 512 : (jc + 1) * 512],
                            start=True,
                            stop=True,
                        )
                    nc.scalar.activation(
                        ea[:, qi * 1024 : (qi + 1) * 1024],
                        eps[:],
                        mybir.ActivationFunctionType.Exp,
                        bias=zero_sb[:, 0:1],
                        accum_out=lpart[:, qi : qi + 1],
                    )

                # tile-0 ea/lpart allocated up front so its energy quarters can
                # be emitted inside the projection sequence
                ea0 = eapool.tile([128, N], BF16, tag="ea", name="ea_t0")
                lpart0 = spool.tile([128, 4], F32, tag="lpart", name="lpart_t0")
                proj_chunk(wq_sb, xq_sb, q_sb, 0, 0, "qproj0")
                for g in range(4):
                    proj_chunk(wk_sb, x_sb, k_sb, 2 * g, 1, f"kproj{2*g}")
                    proj_chunk(wk_sb, x_sb, k_sb, 2 * g + 1, 1, f"kproj{2*g+1}")
                    if g > 0:
                        proj_chunk(wq_sb, xq_sb, q_sb, g, 0, f"qproj{g}")
                    energy_quarter(0, g, ea0, lpart0)

                # xt/xqt are first needed by tile-0 AV; chain them behind the
                # x loads so they do not steal early DMA bandwidth.
                xt_sb = cpool.tile([128, NCHUNK, C], BF16)
                i1 = nc.sync.dma_start(
                    xt_sb[:], xt_d.rearrange("p (j c) -> p j c", j=NCHUNK)
                )
                bass._add_dep_helper(i1.ins, last_in_dma[0].ins, False,
                                     "xt load after x loads")
                xqt_sb = cpool.tile([128, NT, C], BF16)
                i2 = nc.scalar.dma_start(
                    xqt_sb[:], xqt_d.rearrange("p (t c) -> p t c", t=NT)
                )

                # ---- main loop over q-tiles, software-pipelined ----
                # PE stream per iteration: T(t) -> E(t+1) -> AV(t); the E(t+1)
                # matmuls give the DVE copies of T(t) time to land before
                # AV(t) needs them.
                eas = {0: (ea0, lpart0)}

                def emit_energy_tile(t):
                    ea = eapool.tile([128, N], BF16, tag="ea", name=f"ea_t{t}")
                    lpart = spool.tile([128, 4], F32, tag="lpart",
                                       name=f"lpart_t{t}")
                    for qi in range(4):
                        energy_quarter(t, qi, ea, lpart)
                    eas[t] = (ea, lpart)

                emit_energy_tile(1)
                for t in range(NT):
                    qs = t * 128
                    ea, lpart = eas.pop(t)
                    # transpose exp values (bf16) for the AV contraction.
                    # Emitted before the normalize so the DVE runs the copies
                    # (which gate PE's AV matmuls) ahead of the long normalize.
                    at_sb = atpool.tile([128, NCHUNK * 128], BF16, tag="at")
                    for g in range(4):
                        tps = t_ps.tile([128, 1024], BF16, tag="tr")
                        for i in range(8):
                            j = g * 8 + i
                            nc.tensor.transpose(
                                tps[:, i * 128 : (i + 1) * 128],
                                ea[:, j * 128 : (j + 1) * 128],
                                ident[:],
                            )
                        nc.vector.tensor_copy(
                            at_sb[:, g * 1024 : (g + 1) * 1024], tps[:]
                        )
                    if t + 2 < NT:
                        emit_energy_tile(t + 2)
                    # AV: out~^T[m, c] = sum_n A~T[n, m] x^T[n, c]
                    ops = o_ps.tile([128, C], F32, tag="av")
                    for j in range(NCHUNK):
                        nc.tensor.matmul(
                            ops[:],
                            at_sb[:, j * 128 : (j + 1) * 128],
                            xt_sb[:, j, :],
                            start=(j == 0),
                            stop=(j == NCHUNK - 1),
                        )
                    # softmax scales
                    l_sum = spool.tile([128, 1], F32, tag="lsum")
                    nc.vector.reduce_sum(l_sum[:], lpart[:], axis=mybir.AxisListType.X)
                    inv_l = spool.tile([128, 1], F32, tag="invl")
                    nc.vector.reciprocal(inv_l[:], l_sum[:])
                    ginv = spool.tile([128, 1], F32, tag="ginv")
                    nc.vector.tensor_mul(ginv[:], inv_l[:], bg_sb[:, 2:3])
                    # normalized attention row block -> DRAM
                    attn_sb = attnpool.tile([128, N], F32, tag="attn")
                    nc.vector.tensor_scalar(
                        out=attn_sb[:],
                        in0=ea[:],
                        scalar1=inv_l[:, 0:1],
                        scalar2=None,
                        op0=mybir.AluOpType.mult,
                    )
                    nc.sync.dma_start(attn_d[qs : qs + 128, :], attn_sb[:])
                    # epilogue: outt = (gamma/l) * out~^T + xq^T
                    outt_sb = outpool.tile([128, C], F32, tag="outt")
                    av_bf = outpool.tile([128, C], BF16, tag="avbf")
                    # on ScalarE: frees the AV PSUM slot without queuing behind
                    # the normalize on the vector engine
                    nc.scalar.activation(
                        av_bf[:],
                        ops[:],
                        mybir.ActivationFunctionType.Copy,
                        bias=0.0,
                        scale=ginv[:, 0:1],
                    )
                    nc.vector.tensor_add(outt_sb[:], av_bf[:], xqt_sb[:, t, :])
                    nc.sync.dma_start(outt_d[qs : qs + 128, :], outt_sb[:])

    nc.compile()
    return nc


def _prep_inputs(x, w_q, b_q, w_k, b_k, gamma):
    xf = np.ascontiguousarray(x.reshape(B, C, N)).astype(np.float32)
    wqt = np.ascontiguousarray(w_q.T)  # [C, CQ]
    wkt = np.ascontiguousarray(w_k.T)
    # [128, 2*CQ]: row p holds wqt[p, :] then wqt[p+128, :]
    wqt_r = wqt.reshape(2, 128, CQ).transpose(1, 0, 2)  # [128, 2, CQ]
    wkt_r = wkt.reshape(2, 128, CQ).transpose(1, 0, 2)
    wqt_r = np.concatenate([wqt_r, wqt_r], axis=2).reshape(128, 4 * CQ)
    wkt_r = np.concatenate([wkt_r, wkt_r], axis=2).reshape(128, 4 * CQ)
    wqt_r = np.ascontiguousarray(wqt_r).astype(BF16_NP)
    wkt_r = np.ascontiguousarray(wkt_r).astype(BF16_NP)
    bg = np.zeros((128, 3), np.float32)
    bg[:CQ, 0] = b_q
    bg[CQ:2 * CQ, 0] = b_q
    bg[:CQ, 1] = b_k
    bg[CQ:2 * CQ, 1] = b_k
    bg[:, 2] = gamma[0]

    in_maps = []
    for b in range(B):
        xb = xf[b]  # [C, N]
        xtb = np.ascontiguousarray(xb.T)  # [N, C] f32
        xt_r = (
            xtb.reshape(NCHUNK, 128, C)
            .transpose(1, 0, 2)
            .reshape(128, NCHUNK * C)
            .astype(BF16_NP)
        )
        for h in range(2):
            xq = np.ascontiguousarray(xb[:, h * NQ : (h + 1) * NQ])
            xqt = np.ascontiguousarray(xq.T)  # [NQ, C]
            xqt_r = (
                xqt.reshape(NT, 128, C)
                .transpose(1, 0, 2)
                .reshape(128, NT * C)
                .astype(BF16_NP)
            )
            in_maps.append(
                {
                    "x": xb.astype(BF16_NP),
                    "xq": xq.astype(BF16_NP),
                    "xt": xt_r,
                    "xqt": xqt_r,
                    "wqt": wqt_r,
                    "wkt": wkt_r,
                    "bg": bg,
                }
            )
    return in_maps


def run_on_device(x, w_q, b_q, w_k, b_k, gamma, trace=False, tmpdir=None):
    if "nc" not in _CACHED:
        _CACHED["nc"] = build_graph()
    nc = _CACHED["nc"]
    in_maps = _prep_inputs(x, w_q, b_q, w_k, b_k, gamma)
    res = run_bass_kernel_spmd(
        nc, in_maps, core_ids=list(range(8)), trace=trace, tmpdir=tmpdir
    )
    out = np.empty((B, C, N), np.float32)
    attention = np.empty((B, N, N), np.float32)
    for core in range(8):
        b, h = divmod(core, 2)
        r = res.results[core]
        attention[b, h * NQ : (h + 1) * NQ, :] = r["attn"]
        out[b][:, h * NQ : (h + 1) * NQ] = r["outt"].T
    return out.reshape(B, C, W, H), attention, res


def kernel(x, w_q, b_q, w_k, b_k, gamma):
    out, attention, _ = run_on_device(x, w_q, b_q, w_k, b_k, gamma)
    return out, attention


# revision 20
# speedup vs baseline: 1.1274x; 1.0028x over previous
"""SAGAN-style self-attention on 8 TRN2 NeuronCores.

Reference computes, per batch b (B=4, C=256, N=64*64=4096, Cq=64):
    q = w_q @ x + b_q            [Cq, N]
    k = w_k @ x + b_k            [Cq, N]
    energy = q^T k               [N, N]
    attention = softmax_j(energy)
    out = gamma * (x @ attention^T) + x
and returns (out, attention).

Sharding: 8 cores = 4 batches x 2 query-row halves (2048 rows each).
Each core computes its [2048, 4096] attention slice and [256, 2048]
output slice; no collectives needed. Host gathers/concatenates.

Per-core device pipeline (16 q-tiles of 128 rows):
  energy tile   : PE matmul fp32r (1 cyc/row), q-tile stationary
  exp + rowsum  : ScalarE activation Exp with accum_out (no max pass;
                  |energy| <~ 6 so exp is fp32-safe unnormalized)
  normalize     : VectorE tensor_scalar (bf16 src, 2x mode) -> f32 attn
  transpose     : PE bf16 transpose of exp values (for the AV contract)
  AV            : PE bf16 matmul, x^T moving; gamma/l folded into epilogue
"""

import sys

sys.path.insert(0, "/opt/trn_rl_repo")

import numpy as np
import ml_dtypes

import concourse.bass as bass
import concourse.bacc as bacc
import concourse.mybir as mybir
from concourse import tile
from concourse.bass_utils import run_bass_kernel_spmd
from concourse.masks import make_identity

B, C, W, H = 4, 256, 64, 64
N = W * H          # 4096 tokens
CQ = C // 4        # 64
NQ = N // 2        # 2048 query rows per core
NT = NQ // 128     # 16 q-tiles per core
NJ = N // 512      # 8 key chunks of 512
NCHUNK = N // 128  # 32 transpose chunks

F32 = mybir.dt.float32
F32R = mybir.dt.float32r
BF16 = mybir.dt.bfloat16
BF16_NP = ml_dtypes.bfloat16

_CACHED = {}


def build_graph(stages="ABCD"):
    import os
    stages = os.environ.get("K_STAGES", stages)
    nc = bacc.Bacc("TRN2", target_bir_lowering=False, debug=False, num_devices=8)

    x_d = nc.dram_tensor("x", [C, N], BF16, kind="ExternalInput").ap()
    xq_d = nc.dram_tensor("xq", [C, NQ], BF16, kind="ExternalInput").ap()
    xt_d = nc.dram_tensor("xt", [128, NCHUNK * C], BF16, kind="ExternalInput").ap()
    xqt_d = nc.dram_tensor("xqt", [128, NT * C], BF16, kind="ExternalInput").ap()
    wqt_d = nc.dram_tensor("wqt", [128, 4 * CQ], BF16, kind="ExternalInput").ap()
    wkt_d = nc.dram_tensor("wkt", [128, 4 * CQ], BF16, kind="ExternalInput").ap()
    bg_d = nc.dram_tensor("bg", [128, 3], F32, kind="ExternalInput").ap()

    attn_d = nc.dram_tensor("attn", [NQ, N], F32, kind="ExternalOutput").ap()
    outt_d = nc.dram_tensor("outt", [NQ, C], F32, kind="ExternalOutput").ap()

    with tile.TileContext(nc) as tc:
        with (
            tc.tile_pool(name="const", bufs=1) as cpool,
            tc.tile_pool(name="kq", bufs=1) as kqpool,
            tc.tile_pool(name="ea", bufs=3) as eapool,
            tc.tile_pool(name="attn", bufs=2) as attnpool,
            tc.tile_pool(name="at", bufs=3) as atpool,
            tc.tile_pool(name="small", bufs=4) as spool,
            tc.tile_pool(name="outsb", bufs=2) as outpool,
        ):
            # ---- constants / full-lifetime inputs ----
            ident = cpool.tile([128, 128], BF16)
            make_identity(nc, ident[:])

            # Scalar-engine DMAs go on the qActDynamicHW ring, parallel to
            # sync's qSPDynamicHW ring — halves prologue DMA issue latency.
            wq_sb = cpool.tile([128, 2, 2 * CQ], BF16)
            nc.scalar.dma_start(wq_sb[:], wqt_d.rearrange("p (c o) -> p c o", c=2))
            wk_sb = cpool.tile([128, 2, 2 * CQ], BF16)
            nc.scalar.dma_start(wk_sb[:], wkt_d.rearrange("p (c o) -> p c o", c=2))
            bg_sb = cpool.tile([128, 3], F32)
            nc.scalar.dma_start(bg_sb[:], bg_d[:])
            zero_sb = cpool.tile([128, 1], F32)
            nc.gpsimd.memset(zero_sb[:], 0.0)
            # Preload the Exp table off the critical path.
            scratch1 = cpool.tile([128, 1], BF16)
            nc.scalar.activation(
                scratch1[:], zero_sb[:, 0:1],
                mybir.ActivationFunctionType.Exp, bias=zero_sb[:, 0:1],
            )

            x_sb = [cpool.tile([128, N], BF16, tag=f"x{i}", name=f"x_sb{i}") for i in range(2)]
            xq_sb = [cpool.tile([128, NQ], BF16, tag=f"xq{i}", name=f"xq_sb{i}") for i in range(2)]

            last_in_dma = [None]

            # c-half 0 rides the sync HW-DGE ring, c-half 1 the scalar ring:
            # the two halves of each column-quarter (both needed by its
            # k-projection) arrive in parallel.
            def load_x(ci, q):
                eng = nc.sync if ci == 0 else nc.scalar
                last_in_dma[0] = eng.dma_start(
                    x_sb[ci][:, q * 1024 : (q + 1) * 1024],
                    x_d[ci * 128 : (ci + 1) * 128, q * 1024 : (q + 1) * 1024],
                )

            def load_xq(ci, hh):
                eng = nc.sync if ci == 0 else nc.scalar
                eng.dma_start(
                    xq_sb[ci][:, hh * 1024 : (hh + 1) * 1024],
                    xq_d[ci * 128 : (ci + 1) * 128, hh * 1024 : (hh + 1) * 1024],
                )

            # Need-order: q-proj chunk 0 (xq h0) and k-proj 0/1 (x q0) gate
            # tile-0 energy; everything else trails.
            load_xq(0, 0); load_xq(1, 0)
            load_x(0, 0); load_x(1, 0)
            load_xq(0, 1); load_xq(1, 1)
            load_x(0, 1); load_x(1, 1)
            load_x(0, 2); load_x(1, 2)
            load_x(0, 3); load_x(1, 3)

            # ---- projections + tile-0 energy, interleaved in need-order ----
            # k/q live duplicated in both partition halves (rows 0-63 and
            # 64-127) so consecutive energy matmuls can alternate PE
            # row-groups, letting LDWEIGHTS overlap in-flight matmuls.
            k_sb = kqpool.tile([128, N], BF16)
            q_sb = kqpool.tile([128, NQ], BF16)
            with (
                tc.tile_pool(name="e_ps", bufs=2, space="PSUM") as e_ps,
                tc.tile_pool(name="t_ps", bufs=2, space="PSUM") as t_ps,
                tc.tile_pool(name="o_ps", bufs=2, space="PSUM") as o_ps,
            ):
                proj_ps = o_ps  # share the AV pool's banks (free in prologue)
                def proj_chunk(w_sb, src_sb, dst, jc, bias_col, name):
                    ps = proj_ps.tile([128, 512], F32, tag="av", name=name)
                    for cc in range(2):
                        nc.tensor.matmul(
                            ps[:],
                            w_sb[:, cc, :],
                            src_sb[cc][:, jc * 512 : (jc + 1) * 512],
                            start=(cc == 0),
                            stop=(cc == 1),
                        )
                    nc.vector.tensor_scalar(
                        out=dst[:, jc * 512 : (jc + 1) * 512],
                        in0=ps[:],
                        scalar1=bg_sb[:, bias_col : bias_col + 1],
                        scalar2=None,
                        op0=mybir.AluOpType.add,
                    )

                def energy_quarter(t, qi, ea, lpart):
                    qs = t * 128
                    eps = e_ps.tile([128, 1024], F32, tag="energy", name=f"e{t}_{qi}")
                    for jj in range(2):
                        jc = qi * 2 + jj
                        rh = slice(0, CQ) if jc % 2 == 0 else slice(CQ, 128)
                        nc.tensor.matmul(
                            eps[:, jj * 512 : (jj + 1) * 512],
                            q_sb[rh, qs : qs + 128],
                            k_sb[rh, jc * 512 : (jc + 1) * 512],
                            start=True,
                            stop=True,
                        )
                    nc.scalar.activation(
                        ea[:, qi * 1024 : (qi + 1) * 1024],
                        eps[:],
                        mybir.ActivationFunctionType.Exp,
                        bias=zero_sb[:, 0:1],
                        accum_out=lpart[:, qi : qi + 1],
                    )

                # tile-0 ea/lpart allocated up front so its energy quarters can
                # be emitted inside the projection sequence
                ea0 = eapool.tile([128, N], BF16, tag="ea", name="ea_t0")
                lpart0 = spool.tile([128, 4], F32, tag="lpart", name="lpart_t0")
                proj_chunk(wq_sb, xq_sb, q_sb, 0, 0, "qproj0")
                for g in range(4):
                    proj_chunk(wk_sb, x_sb, k_sb, 2 * g, 1, f"kproj{2*g}")
                    proj_chunk(wk_sb, x_sb, k_sb, 2 * g + 1, 1, f"kproj{2*g+1}")
                    if g > 0:
                        proj_chunk(wq_sb, xq_sb, q_sb, g, 0, f"qproj{g}")
                    energy_quarter(0, g, ea0, lpart0)

                # xt/xqt are first needed by tile-0 AV; chain them behind the
                # x loads so they do not steal early DMA bandwidth.
                xt_sb = cpool.tile([128, NCHUNK, C], BF16)
                i1 = nc.sync.dma_start(
                    xt_sb[:], xt_d.rearrange("p (j c) -> p j c", j=NCHUNK)
                )
                bass._add_dep_helper(i1.ins, last_in_dma[0].ins, False,
                                     "xt load after x loads")
                xqt_sb = cpool.tile([128, NT, C], BF16)
                i2 = nc.scalar.dma_start(
                    xqt_sb[:], xqt_d.rearrange("p (t c) -> p t c", t=NT)
                )

                # ---- main loop over q-tiles, software-pipelined ----
                # PE stream per iteration: T(t) -> E(t+1) -> AV(t); the E(t+1)
                # matmuls give the DVE copies of T(t) time to land before
                # AV(t) needs them.
                eas = {0: (ea0, lpart0)}

                def emit_energy_tile(t):
                    ea = eapool.tile([128, N], BF16, tag="ea", name=f"ea_t{t}")
                    lpart = spool.tile([128, 4], F32, tag="lpart",
                                       name=f"lpart_t{t}")
                    for qi in range(4):
                        energy_quarter(t, qi, ea, lpart)
                    eas[t] = (ea, lpart)

                emit_energy_tile(1)
                for t in range(NT):
                    qs = t * 128
                    ea, lpart = eas.pop(t)
                    # transpose exp values (bf16) for the AV contraction.
                    # Emitted before the normalize so the DVE runs the copies
                    # (which gate PE's AV matmuls) ahead of the long normalize.
                    at_sb = atpool.tile([128, NCHUNK * 128], BF16, tag="at")
                    for g in range(4):
                        tps = t_ps.tile([128, 1024], BF16, tag="tr")
                        for i in range(8):
                            j = g * 8 + i
                            nc.tensor.transpose(
                                tps[:, i * 128 : (i + 1) * 128],
                                ea[:, j * 128 : (j + 1) * 128],
                                ident[:],
                            )
                        nc.vector.tensor_copy(
                            at_sb[:, g * 1024 : (g + 1) * 1024], tps[:]
                        )
                    if t + 2 < NT:
                        emit_energy_tile(t + 2)
                    # AV: out~^T[m, c] = sum_n A~T[n, m] x^T[n, c]
                    ops = o_ps.tile([128, C], F32, tag="av")
                    for j in range(NCHUNK):
                        nc.tensor.matmul(
                            ops[:],
                            at_sb[:, j * 128 : (j + 1) * 128],
                            xt_sb[:, j, :],
                            start=(j == 0),
                            stop=(j == NCHUNK - 1),
                        )
                    # softmax scales
                    l_sum = spool.tile([128, 1], F32, tag="lsum")
                    nc.vector.reduce_sum(l_sum[:], lpart[:], axis=mybir.AxisListType.X)
                    inv_l = spool.tile([128, 1], F32, tag="invl")
                    nc.vector.reciprocal(inv_l[:], l_sum[:])
                    ginv = spool.tile([128, 1], F32, tag="ginv")
                    nc.vector.tensor_mul(ginv[:], inv_l[:], bg_sb[:, 2:3])
                    # normalized attention row block -> DRAM (last tiles
                    # split in half so the final DMA drains sooner)
                    attn_sb = attnpool.tile([128, N], F32, tag="attn")
                    n_split = 2 if t >= NT - 2 else 1
                    for hh in range(n_split):
                        sl = slice(hh * (N // n_split), (hh + 1) * (N // n_split))
                        nc.vector.tensor_scalar(
                            out=attn_sb[:, sl],
                            in0=ea[:, sl],
                            scalar1=inv_l[:, 0:1],
                            scalar2=None,
                            op0=mybir.AluOpType.mult,
                        )
                        nc.sync.dma_start(attn_d[qs : qs + 128, sl], attn_sb[:, sl])
                    # epilogue: outt = (gamma/l) * out~^T + xq^T
                    outt_sb = outpool.tile([128, C], F32, tag="outt")
                    av_bf = outpool.tile([128, C], BF16, tag="avbf")
                    # on ScalarE: frees the AV PSUM slot without queuing behind
                    # the normalize on the vector engine
                    nc.scalar.activation(
                        av_bf[:],
                        ops[:],
                        mybir.ActivationFunctionType.Copy,
                        bias=0.0,
                        scale=ginv[:, 0:1],
                    )
                    nc.vector.tensor_add(outt_sb[:], av_bf[:], xqt_sb[:, t, :])
                    nc.sync.dma_start(outt_d[qs : qs + 128, :], outt_sb[:])

    nc.compile()
    return nc


def _prep_inputs(x, w_q, b_q, w_k, b_k, gamma):
    xf = np.ascontiguousarray(x.reshape(B, C, N)).astype(np.float32)
    wqt = np.ascontiguousarray(w_q.T)  # [C, CQ]
    wkt = np.ascontiguousarray(w_k.T)
    # [128, 2*CQ]: row p holds wqt[p, :] then wqt[p+128, :]
    wqt_r = wqt.reshape(2, 128, CQ).transpose(1, 0, 2)  # [128, 2, CQ]
    wkt_r = wkt.reshape(2, 128, CQ).transpose(1, 0, 2)
    wqt_r = np.concatenate([wqt_r, wqt_r], axis=2).reshape(128, 4 * CQ)
    wkt_r = np.concatenate([wkt_r, wkt_r], axis=2).reshape(128, 4 * CQ)
    wqt_r = np.ascontiguousarray(wqt_r).astype(BF16_NP)
    wkt_r = np.ascontiguousarray(wkt_r).astype(BF16_NP)
    bg = np.zeros((128, 3), np.float32)
    bg[:CQ, 0] = b_q
    bg[CQ:2 * CQ, 0] = b_q
    bg[:CQ, 1] = b_k
    bg[CQ:2 * CQ, 1] = b_k
    bg[:, 2] = gamma[0]

    in_maps = []
    for b in range(B):
        xb = xf[b]  # [C, N]
        xtb = np.ascontiguousarray(xb.T)  # [N, C] f32
        xt_r = (
            xtb.reshape(NCHUNK, 128, C)
            .transpose(1, 0, 2)
            .reshape(128, NCHUNK * C)
            .astype(BF16_NP)
        )
        for h in range(2):
            xq = np.ascontiguousarray(xb[:, h * NQ : (h + 1) * NQ])
            xqt = np.ascontiguousarray(xq.T)  # [NQ, C]
            xqt_r = (
                xqt.reshape(NT, 128, C)
                .transpose(1, 0, 2)
                .reshape(128, NT * C)
                .astype(BF16_NP)
            )
            in_maps.append(
                {
                    "x": xb.astype(BF16_NP),
                    "xq": xq.astype(BF16_NP),
                    "xt": xt_r,
                    "xqt": xqt_r,
                    "wqt": wqt_r,
                    "wkt": wkt_r,
                    "bg": bg,
                }
            )
    return in_maps


def run_on_device(x, w_q, b_q, w_k, b_k, gamma, trace=False, tmpdir=None):
    if "nc" not in _CACHED:
        _CACHED["nc"] = build_graph()
    nc = _CACHED["nc"]
    in_maps = _prep_inputs(x, w_q, b_q, w_k, b_k, gamma)
    res = run_bass_kernel_spmd(
        nc, in_maps, core_ids=list(range(8)), trace=trace, tmpdir=tmpdir
    )
    out = np.empty((B, C, N), np.float32)
    attention = np.empty((B, N, N), np.float32)
    for core in range(8):
        b, h = divmod(core, 2)
        r = res.results[core]
        attention[b, h * NQ : (h + 1) * NQ, :] = r["attn"]
        out[b][:, h * NQ : (h + 1) * NQ] = r["outt"].T
    return out.reshape(B, C, W, H), attention, res


def kernel(x, w_q, b_q, w_k, b_k, gamma):
    out, attention, _ = run_on_device(x, w_q, b_q, w_k, b_k, gamma)
    return out, attention
